# revision 52
# baseline (speedup 1.0000x reference)
"""MFGCGRU (graph-conv GRU cell) Trainium2 kernel.

Strategy: data-parallel over batch B=32 across 8 NeuronCores (4 batches
per core), NxN supports replicated. The diffusion conv is kernel-first:
S_m @ (X @ k_m), with the node contractions run as fp8e4m3 DoubleRow
matmuls (2 K-blocks per instruction at 0.5 cycles/row = 4x bf16 MAC
throughput). fp8's narrow exponent range is handled by host-side
power-of-two scaling:

  - adjacency S^T stored fp8 at x64,
  - Y = X @ (8 x kernel) quantized to fp8 (so adj-terms come out x512),
  - identity-path kernels kk0/kc0 stored bf16 at x512,
  - the attention support stays raw in fp8 (e = exp(64*QK/8) written by
    ACT straight off the QK PSUM, itself an fp8 DoubleRow matmul over
    u-halves). Its normalizer rdbc = 64/(s + rowsum(e)) is produced as a
    full [128, n] broadcast by a DoubleRow colsum against a constant 1/64
    tile (the sentinel s rides along as row 0 of a zero tile), and each
    gate group contracts e FIRST into its PSUM bank, multiplies the
    partial by rdbc on DVE, then accumulates identity + adjacency terms
    on top (x8 y-scale x64 adj-scale = x512 everywhere).
  - gates read PSUM directly: both sigmoids are evaluated as
    0.5 + 0.5*tanh(z/2) so Relu/Exp/Tanh/Copy all live in one ACT
    function table (a single LoadActFuncSet for the whole kernel); the
    0.5s fold into the c-kernels' h-rows and the GRU tail's fused
    scalar_tensor_tensor ops.

e (4.2MB) and both adjacency operands (8.4MB) stay resident in SBUF so
exp and the adjacency DMAs run once across both passes. PSUM->SBUF
evacuations are balanced across DVE and ACT; the SBUF-only elementwise
work (r*h, GRU tail subtract) runs on the otherwise idle Pool engine.
"""

import contextlib
import os

import numpy as np
import ml_dtypes

import concourse.bass as bass
import concourse.bacc as bacc
import concourse.tile as tile
from concourse import mybir
from concourse.bass_utils import run_bass_kernel_spmd

F32 = mybir.dt.float32
BF16 = mybir.dt.bfloat16
FP8 = mybir.dt.float8e4
AF = mybir.ActivationFunctionType
DR = mybir.MatmulPerfMode.DoubleRow

B, N, DIN, U, FD, SD = 32, 2048, 2, 64, 32, 64
NCORES = 8
BL = B // NCORES          # batches per core
NTW = 512                 # n-tile width
NT = N // NTW             # 4 n-tiles
NBW = 128                 # node-block width
NB = N // NBW             # 16 node blocks
NP = NB // 2              # 8 node-block pairs (DoubleRow)
FROWS = DIN + U           # 66

SC_ADJ = 64.0             # host scale on adjacency (fp8)
SC_Y = 8.0                # host scale on y kernels (fp8 y tiles)
SC_ID = 512.0             # host scale on identity kernels (bf16)
GATE_SCALE = 0.25 / 512.0 # sigmoid/tanh pre-scale: mean over 4 supports / 512


def _build_program():
    nc = bacc.Bacc("TRN2", debug=False, num_devices=NCORES)

    d = {}

    def din(name, shape, dt):
        d[name] = nc.dram_tensor(name, shape, dt, kind="ExternalInput").ap()

    din("xT", [BL, FROWS, N], BF16)
    din("hT", [BL, U, N], F32)
    din("a1T", [N, N], FP8)
    din("a2T", [N, N], FP8)
    din("fsT", [FD + SD, N], F32)
    din("wq", [FD, U], F32)
    din("wk", [FD, U], F32)
    din("ws1", [FD + SD, U], F32)
    din("bs1v", [U, 1], F32)
    din("ws2", [U, 1], F32)
    din("bs2v", [1, 1], F32)
    din("kkall", [FROWS, 3 * 2 * U], BF16)
    din("kk0", [FROWS, 2 * U], BF16)
    din("kcall", [FROWS, 3 * U], BF16)
    din("kc0", [FROWS, U], BF16)
    din("bruh", [2 * U, 1], F32)
    din("bc2", [2 * U, 1], F32)
    din("ones8", [NBW, 2, NBW], FP8)        # constant 1/64
    out_h = nc.dram_tensor("out", [BL, U, N], F32, kind="ExternalOutput").ap()
    uscr = nc.dram_tensor("uscr", [BL, U, N], F32).ap()

    with tile.TileContext(nc) as tc, \
            nc.allow_low_precision(reason="fp8 support contraction by design"):
        _emit(tc, d, out_h, uscr)
    nc.compile()
    return nc


def _emit(tc, d, out_h, uscr):
    nc = tc.nc
    ctx = contextlib.ExitStack()
    const = ctx.enter_context(tc.tile_pool(name="const", bufs=1))
    persist = ctx.enter_context(tc.tile_pool(name="persist", bufs=1))
    adjp = ctx.enter_context(tc.tile_pool(name="adjp", bufs=4))
    ypool = ctx.enter_context(tc.tile_pool(name="ypool", bufs=1))
    stage = ctx.enter_context(tc.tile_pool(name="stage", bufs=2))
    p3p = ctx.enter_context(tc.tile_pool(name="p3p", bufs=2))
    psacc = ctx.enter_context(tc.tile_pool(name="psacc", bufs=3, space="PSUM"))
    psscr = ctx.enter_context(tc.tile_pool(name="psscr", bufs=3, space="PSUM"))
    pscs = ctx.enter_context(tc.tile_pool(name="pscs", bufs=2, space="PSUM"))

    # ---- constants / weights in SBUF ----
    def cload(name):
        ap = d[name]
        t = const.tile(list(ap.shape), ap.dtype, name=f"c_{name}")
        nc.sync.dma_start(out=t, in_=ap)
        return t

    fsT = const.tile([FD + SD, N], F32, name="c_fsT")
    nc.sync.dma_start(out=fsT[0:FD, :], in_=d["fsT"][0:FD, :])
    wq = cload("wq")
    wk = cload("wk")
    nc.sync.dma_start(out=fsT[FD:, :], in_=d["fsT"][FD:, :])
    ws1 = cload("ws1")
    bs1v = cload("bs1v")
    ws2 = cload("ws2")
    bs2v = cload("bs2v")

    kkall = cload("kkall")
    kcall = cload("kcall")
    kk0 = cload("kk0")
    kc0 = cload("kc0")
    bruh = cload("bruh")
    bc2 = cload("bc2")
    ones8 = cload("ones8")
    s8 = const.tile([NBW, 2, N], FP8, name="s8")
    nc.gpsimd.memset(s8, 0.0)
    one_bc = const.tile([128, NTW], F32, name="one_bc")
    nc.vector.memset(one_bc, 1.0)

    # ---- persistent activations ----
    xT = [persist.tile([FROWS, N], BF16, name=f"xT{b}", tag=f"xT{b}")
          for b in range(BL)]
    for b in range(BL):
        nc.sync.dma_start(out=xT[b], in_=d["xT"][b])

    QT = persist.tile([U // 2, 2, N], FP8, name="QT", tag="QT")
    KT = persist.tile([U // 2, 2, N], FP8, name="KT", tag="KT")
    # resident raw attention support e^T = exp(KQ^T/8), fp8
    et = [persist.tile([NBW, NB, NTW], FP8, name=f"et{t}", tag=f"et{t}")
          for t in range(NT)]
    # rdbc[t][p, n] = 64/d[n]: e-term normalizer, applied to PSUM e-partials
    rdbc = [persist.tile([NBW, NTW], F32, name=f"rdbc{t}", tag=f"rdbc{t}")
            for t in range(NT)]

    # ---- prelude: Q^T, K^T, s ----
    for t in range(NT):
        sl = slice(t * NTW, (t + 1) * NTW)
        # Q/K/s evacuations all on DVE (tensor_scalar relu): keeps the ACT
        # queue free for the 16 serial exp instructions that gate tile 0
        pq = psscr.tile([U, NTW], F32, name="pq", tag="scr")
        nc.tensor.matmul(pq, wq, fsT[0:FD, sl], start=True, stop=True)
        nc.vector.tensor_scalar(QT[:, 0, sl], pq[0:U // 2, :], 8.0, 0.0,
                                mybir.AluOpType.mult, mybir.AluOpType.max)
        nc.scalar.activation(QT[:, 1, sl], pq[U // 2:U, :], AF.Relu,
                             scale=8.0)
        pk = psscr.tile([U, NTW], F32, name="pk", tag="scr")
        nc.tensor.matmul(pk, wk, fsT[0:FD, sl], start=True, stop=True)
        nc.vector.tensor_scalar(KT[:, 0, sl], pk[0:U // 2, :], 8.0, 0.0,
                                mybir.AluOpType.mult, mybir.AluOpType.max)
        nc.scalar.activation(KT[:, 1, sl], pk[U // 2:U, :], AF.Relu,
                             scale=8.0)
        ps1 = psscr.tile([U, NTW], F32, name="ps1", tag="scr")
        nc.tensor.matmul(ps1, ws1, fsT[:, sl], start=True, stop=True)
        s1t = stage.tile([U, NTW], F32, name="s1t", tag="sig")
        nc.vector.tensor_scalar(s1t, ps1, bs1v, 0.0,
                                mybir.AluOpType.add, mybir.AluOpType.max)
        ps2 = psscr.tile([1, NTW], F32, name="ps2", tag="scr")
        nc.tensor.matmul(ps2, ws2, s1t, start=True, stop=True)
        nc.vector.tensor_scalar(s8[0:1, 0, sl], ps2, bs2v, 0.0,
                                mybir.AluOpType.add, mybir.AluOpType.max)

    # ---- e-generation chain (per n-tile t), emitted as thunks ----
    def eg_thunks(t):
        """raw e^T tile: et[t][:, j, :] = exp(K Q^T / 8) as fp8."""
        sl = slice(t * NTW, (t + 1) * NTW)

        def mk(j):
            def f():
                pe = psscr.tile([NBW, NTW], F32, name="pe", tag="scr")
                nc.tensor.matmul(pe, KT[:, :, j * NBW:(j + 1) * NBW],
                                 QT[:, :, sl], start=True, stop=True,
                                 perf_mode=DR)
                nc.scalar.activation(et[t][:, j, :], pe, AF.Exp,
                                     scale=0.125 / 64.0)
            return f
        return [mk(j) for j in range(NB)]

    def colsum_thunks(t):
        """rdbc[t][p, n] = 64 / (s[n] + colsum(e^T)[n]), all partitions."""
        sl = slice(t * NTW, (t + 1) * NTW)
        pcs = pscs.tile([NBW, NTW], F32, name="pcs", tag="pcs")
        th = []
        for k in range(NP):
            def f(k=k):
                nc.tensor.matmul(pcs, ones8, et[t][:, 2 * k:2 * k + 2, :],
                                 start=(k == 0), stop=False, perf_mode=DR)
            th.append(f)
        th.append(lambda: nc.tensor.matmul(
            pcs, ones8, s8[:, :, sl], start=False, stop=True, perf_mode=DR))
        th.append(lambda: nc.vector.reciprocal(rdbc[t], pcs))
        return th

    # ---- phase-1 Y tiles: Y[m,b] = X_b @ [8*k_r[m]|8*k_u[m]] ----
    # stored fp8 [node%128, node//128, m, u']; copies split DVE/ACT
    y = [ypool.tile([NBW, NB, 3, 2 * U], FP8, name=f"y_{b}", tag=f"y{b}")
         for b in range(BL)]

    def ygen_thunks(b):
        def mk(j):
            def f():
                nsl = slice(j * NBW, (j + 1) * NBW)
                py = psscr.tile([NBW, 3 * 2 * U], F32, name="py", tag="scr")
                nc.tensor.matmul(py, xT[b][:, nsl], kkall, start=True,
                                 stop=True)
                if j % 2 == 0:
                    nc.vector.tensor_copy(
                        y[b][:, j, :, :],
                        py.rearrange("p (m u) -> p m u", m=3))
                else:
                    nc.scalar.activation(
                        y[b][:, j, :, :].rearrange("p m u -> p (m u)"),
                        py, AF.Copy)
            return f
        return [mk(j) for j in range(NB)]

    def interleave(main, extra, ratio=2):
        mi = ei = 0
        while mi < len(main) or ei < len(extra):
            for _ in range(ratio):
                if mi < len(main):
                    main[mi](); mi += 1
            if ei < len(extra):
                extra[ei](); ei += 1


    _adj_cache = {}

    def adjslice(name, t):
        # persist: each slice is DMA'd once (phase 1) and reused in phase 2
        if (name, t) not in _adj_cache:
            sl = d[name][:, t * NTW:(t + 1) * NTW]
            a = persist.tile([NBW, NB, NTW], FP8, name=f"{name}_{t}",
                             tag=f"{name}_{t}")
            nc.sync.dma_start(out=a,
                              in_=sl.rearrange("(j p) w -> p j w", p=NBW))
            _adj_cache[(name, t)] = a
        return _adj_cache[(name, t)]

    # =================== phase 1: r & u gates ===================
    # Each gate group: e-support contraction first (own PSUM group), then a
    # DVE multiply by rdbc[t] on the PSUM partial, then identity + adjacency
    # terms accumulate on top (start=False).
    def e_thunks(yt, t, pa):
        th = []
        for k in range(NP):
            def f(k=k):
                nc.tensor.matmul(pa, yt[:, 2 * k:2 * k + 2, 2, :],
                                 et[t][:, 2 * k:2 * k + 2, :],
                                 start=(k == 0), stop=(k == NP - 1),
                                 perf_mode=DR)
            th.append(f)
        return th

    def rest_thunks1(b, t, sl, a1, a2, pa):
        th = [lambda: nc.vector.tensor_mul(pa, pa, rdbc[t]),
              lambda: nc.tensor.matmul(pa, kk0, xT[b][:, sl], start=False,
                                       stop=False, skip_group_check=True)]
        for m, mov in ((0, a1), (1, a2)):
            for k in range(NP):
                def f(m=m, mov=mov, k=k):
                    nc.tensor.matmul(pa, y[b][:, 2 * k:2 * k + 2, m, :],
                                     mov[:, 2 * k:2 * k + 2, :],
                                     start=False,
                                     stop=(m == 1 and k == NP - 1),
                                     perf_mode=DR, skip_group_check=True)
                th.append(f)
        return th

    def gate1(b, t, sl, pa):
        def f():
            # sigmoid(z) computed as 0.5 + 0.5*tanh(z/2): keeps every ACT
            # function in the exp_and_others table (no table reloads).
            # sig holds T = tanh(z/2) = 2*sigmoid(z) - 1.
            sig = stage.tile([128, NTW], F32, name="sig", tag="sig")
            nc.scalar.activation(sig, pa, AF.Tanh, scale=GATE_SCALE / 2,
                                 bias=bruh)
            # rows 0:64 in place: (T_r + 1)*h = 2*r*h; the c-pass kernels'
            # h-rows carry a compensating 1/2. u kept as raw T_u.
            rtmp = stage.tile([U, NTW], F32, name="rtmp", tag="rtmp")
            nc.gpsimd.tensor_add(rtmp, sig[0:U, :], one_bc[0:U, :])
            nc.gpsimd.tensor_mul(xT[b][0:U, sl], rtmp, xT[b][0:U, sl])
            nc.sync.dma_start(out=uscr[b][:, sl], in_=sig[U:128, :])
        return [f]

    yc = [None, None]

    def ycgen_thunks(yct, p):
        th = []
        for half in range(2):
            b = 2 * p + half
            usl = slice(half * U, (half + 1) * U)
            for j in range(NB):
                def f(b=b, usl=usl, j=j, yct=yct, half=half):
                    nsl = slice(j * NBW, (j + 1) * NBW)
                    pyc = psscr.tile([NBW, 3 * U], F32, name="pyc", tag="scr")
                    nc.tensor.matmul(pyc, xT[b][:, nsl], kcall,
                                     start=True, stop=True)
                    dst = yct[:, j, :, usl]
                    src = pyc.rearrange("p (m u) -> p m u", m=3)
                    if half == 0:
                        nc.vector.tensor_copy(dst, src)
                    else:
                        nc.scalar.activation(dst, src, AF.Copy)
                th.append(f)
        return th

    # prologue: e-gen + colsum for tile 0; adj preload
    for f in eg_thunks(0) + colsum_thunks(0):
        f()
    preload = (adjslice("a1T", 0), adjslice("a2T", 0))

    for t in range(NT):
        sl = slice(t * NTW, (t + 1) * NTW)
        if t == 0:
            a1, a2 = preload
        else:
            a1 = adjslice("a1T", t)
            a2 = adjslice("a2T", t)
        pa = [psacc.tile([128, NTW], F32, name="pa", tag="acc")
              for _ in range(BL)]
        # schedule: e-streams lead their group's rest by one slot so the
        # DVE rdbc-multiply hides under the next group's e-stream.
        main = []
        if t == 0:
            # y generation woven in so each y[b] is ready just before use
            main += ygen_thunks(0) + ygen_thunks(1)
            main += e_thunks(y[0], t, pa[0])
            main += ygen_thunks(2)
            main += e_thunks(y[1], t, pa[1])
            main += rest_thunks1(0, t, sl, a1, a2, pa[0]) + gate1(0, t, sl, pa[0])
            main += ygen_thunks(3)
            main += e_thunks(y[2], t, pa[2])
            main += rest_thunks1(1, t, sl, a1, a2, pa[1]) + gate1(1, t, sl, pa[1])
            main += e_thunks(y[3], t, pa[3])
            main += rest_thunks1(2, t, sl, a1, a2, pa[2]) + gate1(2, t, sl, pa[2])
            main += rest_thunks1(3, t, sl, a1, a2, pa[3]) + gate1(3, t, sl, pa[3])
        else:
            main += e_thunks(y[0], t, pa[0]) + e_thunks(y[1], t, pa[1])
            main += e_thunks(y[2], t, pa[2])
            main += rest_thunks1(0, t, sl, a1, a2, pa[0]) + gate1(0, t, sl, pa[0])
            main += e_thunks(y[3], t, pa[3])
            main += rest_thunks1(1, t, sl, a1, a2, pa[1]) + gate1(1, t, sl, pa[1])
        post = []
        if t > 0:
            post += rest_thunks1(2, t, sl, a1, a2, pa[2]) + gate1(2, t, sl, pa[2])
            post += rest_thunks1(3, t, sl, a1, a2, pa[3]) + gate1(3, t, sl, pa[3])
        extra = eg_thunks(t + 1) if t + 1 < NT else []
        post_extra = []
        if t == NT - 1:
            # yc[0] (batches 0/1) reads post-rh xT: only legal after gate1(1)
            yc[0] = ypool.tile([NBW, NB, 3, 2 * U], FP8, name="yc_0",
                               tag="y0")
            post_extra = ycgen_thunks(yc[0], 0)
        allm = main + post
        alle = extra
        if post_extra:
            interleave(main, extra, ratio=max(1, len(main) // max(1, len(extra))))
            interleave(post, post_extra, ratio=1)
        else:
            interleave(allm, alle, ratio=max(1, len(allm) // max(1, len(alle))))
        # colsum for t+1 at tile end: its exp deps had the whole tile to run
        if t + 1 < NT:
            for f in colsum_thunks(t + 1):
                f()

    # =================== phase 2+3: c gate & h_new ===================
    yc[1] = ypool.tile([NBW, NB, 3, 2 * U], FP8, name="yc_1", tag="y1")
    a1 = adjslice("a1T", 0)
    a2 = adjslice("a2T", 0)
    for f in ycgen_thunks(yc[1], 1):
        f()

    def hu_prefetch(p, t, sl, hu):
        # single 2-batch DMA each for h and u: [2, 64, w] -> [128, w]
        b0 = 2 * p

        def f():
            hp = p3p.tile([128, NTW], F32, name="hp", tag="hp")
            up = p3p.tile([128, NTW], F32, name="up", tag="up")
            nc.sync.dma_start(
                out=hp, in_=d["hT"][b0:b0 + 2, :, sl].rearrange(
                    "b u w -> (b u) w"))
            nc.sync.dma_start(
                out=up, in_=uscr[b0:b0 + 2, :, sl].rearrange(
                    "b u w -> (b u) w"))
            hu.extend((hp, up))
        return [f]

    def rest_thunks2(p, t, sl, a1, a2, pa, hu):
        b0, b1 = 2 * p, 2 * p + 1

        th = [lambda: nc.vector.tensor_mul(pa, pa, rdbc[t]),
              lambda: nc.tensor.matmul(pa[0:U, :], kc0, xT[b0][:, sl],
                                       start=False, stop=False,
                                       skip_group_check=True),
              lambda: nc.tensor.matmul(pa[U:128, :], kc0, xT[b1][:, sl],
                                       start=False, stop=False,
                                       skip_group_check=True)]
        for m, mov in ((0, a1), (1, a2)):
            for k in range(NP):
                def f(m=m, mov=mov, k=k):
                    nc.tensor.matmul(pa, yc[p][:, 2 * k:2 * k + 2, m, :],
                                     mov[:, 2 * k:2 * k + 2, :],
                                     start=False,
                                     stop=(m == 1 and k == NP - 1),
                                     perf_mode=DR, skip_group_check=True)
                th.append(f)
        return th

    def tail2(p, t, sl, pa, hu, splits=2):
        b0, b1 = 2 * p, 2 * p + 1

        def f():
            hp, up = hu
            ct = stage.tile([128, NTW], F32, name="ct", tag="sig")
            t1 = p3p.tile([128, NTW], F32, name="t1", tag="t1")
            # compute in column chunks so ACT/Pool/DVE stages pipeline, but
            # write out full-width (HWDGE descriptor slots are the scarce
            # resource at the drain)
            for c0 in range(0, NTW, NTW // splits):
                cs = slice(c0, c0 + NTW // splits)
                nc.scalar.activation(ct[:, cs], pa[:, cs], AF.Tanh,
                                     scale=GATE_SCALE, bias=bc2)
                # h_new = c + u*(h-c) with up = T_u = 2u-1:
                # t1 = h-c (Pool); t1 = (T_u+1)*t1; t1 = t1/2 + c (DVE)
                nc.gpsimd.tensor_sub(t1[:, cs], hp[:, cs], ct[:, cs])
                nc.vector.scalar_tensor_tensor(
                    t1[:, cs], up[:, cs], 1.0, t1[:, cs],
                    mybir.AluOpType.add, mybir.AluOpType.mult)
                nc.vector.scalar_tensor_tensor(
                    t1[:, cs], t1[:, cs], 0.5, ct[:, cs],
                    mybir.AluOpType.mult, mybir.AluOpType.add)
                if splits == 4 and c0 + NTW // splits in (NTW // 2, NTW):
                    hs = slice(c0 + NTW // splits - NTW // 2,
                               c0 + NTW // splits)
                    nc.scalar.dma_start(
                        out=out_h[b0:b0 + 2, :,
                                  t * NTW + hs.start:t * NTW + hs.stop]
                        .rearrange("b u w -> (b u) w"),
                        in_=t1[:, hs])
            if splits != 4:
                nc.scalar.dma_start(
                    out=out_h[b0:b0 + 2, :, sl].rearrange("b u w -> (b u) w"),
                    in_=t1)
        return [f]

    prev = []
    for t in range(NT):
        sl = slice(t * NTW, (t + 1) * NTW)
        if t > 0:
            a1 = adjslice("a1T", t)
            a2 = adjslice("a2T", t)
        pa = [psacc.tile([128, NTW], F32, name="pa2", tag="acc")
              for _ in range(BL // 2)]
        hu0, hu1 = [], []
        main = hu_prefetch(0, t, sl, hu0) + prev + hu_prefetch(1, t, sl, hu1)
        main += e_thunks(yc[0], t, pa[0]) + e_thunks(yc[1], t, pa[1])
        main += rest_thunks2(0, t, sl, a1, a2, pa[0], hu0) \
            + tail2(0, t, sl, pa[0], hu0)
        for f in main:
            f()
        # carry p=1's rest into the next tile so its DVE multiply and the
        # tail chain overlap with the next tile's e-streams
        prev = rest_thunks2(1, t, sl, a1, a2, pa[1], hu1) \
            + tail2(1, t, sl, pa[1], hu1, splits=4 if t == NT - 1 else 2)
    for f in prev:
        f()

    ctx.close()


_CACHE = {}


def _get_program():
    if "nc" not in _CACHE:
        _CACHE["nc"] = _build_program()
    return _CACHE["nc"]


def _prep_inputs(inputs, h_prev, adj1, adj2, feat, SE, Wq, Wk, Ws1, bs1, Ws2,
                 bs2, r_kernel, r_bias, u_kernel, u_bias, c_kernel, c_bias):
    bf = ml_dtypes.bfloat16
    f8 = ml_dtypes.float8_e4m3fn
    f32 = np.float32
    perm = list(range(DIN, FROWS)) + list(range(DIN))  # [h(64); inputs(2)]

    h3 = np.asarray(h_prev, f32).reshape(B, N, U)
    hT = np.ascontiguousarray(h3.transpose(0, 2, 1))            # [B, U, N]
    inT = np.asarray(inputs, f32).transpose(0, 2, 1)            # [B, DIN, N]
    xT = np.concatenate([hT, inT], axis=1).astype(bf)           # [B, 66, N]

    rk = np.asarray(r_kernel, f32)[:, perm, :]
    uk = np.asarray(u_kernel, f32)[:, perm, :]
    ck = np.asarray(c_kernel, f32)[:, perm, :]
    kkall = np.concatenate(
        [np.concatenate([rk[m], uk[m]], axis=1) for m in (1, 2, 3)],
        axis=1) * SC_Y                                          # [66, 384]
    kk0 = np.concatenate([rk[0], uk[0]], axis=1) * SC_ID        # [66, 128]
    # c-pass x_cat holds 2*r*h in its h-rows (r comes from the tanh-form
    # sigmoid as T_r + 1); compensate with a 1/2 on the kernels' h-rows.
    ck = ck.copy()
    ck[:, 0:U, :] *= 0.5
    kcall = np.concatenate([ck[1], ck[2], ck[3]], axis=1) * SC_Y
    kc0 = ck[0] * SC_ID

    shared = {
        "a1T": np.ascontiguousarray(np.asarray(adj1, f32).T * SC_ADJ).astype(f8),
        "a2T": np.ascontiguousarray(np.asarray(adj2, f32).T * SC_ADJ).astype(f8),
        "fsT": np.ascontiguousarray(
            np.concatenate([np.asarray(feat, f32).T, np.asarray(SE, f32).T],
                           axis=0)),
        "wq": np.asarray(Wq, f32),
        "wk": np.asarray(Wk, f32),
        "ws1": np.asarray(Ws1, f32),
        "bs1v": np.asarray(bs1, f32).reshape(U, 1),
        "ws2": np.asarray(Ws2, f32).reshape(U, 1),
        "bs2v": np.asarray(bs2, f32).reshape(1, 1),
        "kkall": kkall.astype(bf),
        "kk0": kk0.astype(bf),
        "kcall": kcall.astype(bf),
        "kc0": kc0.astype(bf),
        "bruh": np.concatenate([np.asarray(r_bias, f32).mean(0),
                                np.asarray(u_bias, f32).mean(0)]
                               ).reshape(-1, 1) * 0.5,
        "bc2": np.tile(np.asarray(c_bias, f32).mean(0), 2).reshape(-1, 1),
        "ones8": np.full((NBW, 2, NBW), 1.0 / 64.0, f8),
    }
    in_maps = []
    for c in range(NCORES):
        bsl = slice(c * BL, (c + 1) * BL)
        m = dict(shared)
        m["xT"] = np.ascontiguousarray(xT[bsl])
        m["hT"] = np.ascontiguousarray(hT[bsl])
        in_maps.append(m)
    return in_maps


def kernel(**inputs):
    os.environ.setdefault("NEURON_RT_RESET_CORES", "1")
    nc = _get_program()
    in_maps = _prep_inputs(**inputs)
    res = None
    err = None
    for _ in range(2):
        try:
            res = run_bass_kernel_spmd(nc, in_maps, list(range(NCORES)))
            break
        except Exception as e:  # e.g. a wedged device; retry once
            err = e
    if res is None:
        raise err
    outs = []
    for c in range(NCORES):
        o = res.results[c]["out"]                     # [BL, U, N] f32
        outs.append(o.transpose(0, 2, 1).reshape(BL, N * U))
    return np.concatenate(outs, axis=0).astype(np.float32)


# revision 53
# speedup vs baseline: 1.0154x; 1.0154x over previous
"""MFGCGRU (graph-conv GRU cell) Trainium2 kernel.

Strategy: data-parallel over batch B=32 across 8 NeuronCores (4 batches
per core), NxN supports replicated. The diffusion conv is kernel-first:
S_m @ (X @ k_m), with the node contractions run as fp8e4m3 DoubleRow
matmuls (2 K-blocks per instruction at 0.5 cycles/row = 4x bf16 MAC
throughput). fp8's narrow exponent range is handled by host-side
power-of-two scaling:

  - adjacency S^T stored fp8 at x64,
  - Y = X @ (8 x kernel) quantized to fp8 (so adj-terms come out x512),
  - identity-path kernels kk0/kc0 stored bf16 at x512,
  - the attention support stays raw in fp8 (e = exp(64*QK/8) written by
    ACT straight off the QK PSUM, itself an fp8 DoubleRow matmul over
    u-halves). Its normalizer rdbc = 64/(s + rowsum(e)) is produced as a
    full [128, n] broadcast by a DoubleRow colsum against a constant 1/64
    tile (the sentinel s rides along as row 0 of a zero tile), and each
    gate group contracts e FIRST into its PSUM bank, multiplies the
    partial by rdbc on DVE, then accumulates identity + adjacency terms
    on top (x8 y-scale x64 adj-scale = x512 everywhere).
  - gates read PSUM directly: both sigmoids are evaluated as
    0.5 + 0.5*tanh(z/2) so Relu/Exp/Tanh/Copy all live in one ACT
    function table (a single LoadActFuncSet for the whole kernel); the
    0.5s fold into the c-kernels' h-rows and the GRU tail's fused
    scalar_tensor_tensor ops.

e (4.2MB) and both adjacency operands (8.4MB) stay resident in SBUF so
exp and the adjacency DMAs run once across both passes. PSUM->SBUF
evacuations are balanced across DVE and ACT; the SBUF-only elementwise
work (r*h, GRU tail subtract) runs on the otherwise idle Pool engine.
"""

import contextlib
import os

import numpy as np
import ml_dtypes

import concourse.bass as bass
import concourse.bacc as bacc
import concourse.tile as tile
from concourse import mybir
from concourse.bass_utils import run_bass_kernel_spmd

F32 = mybir.dt.float32
BF16 = mybir.dt.bfloat16
FP8 = mybir.dt.float8e4
AF = mybir.ActivationFunctionType
DR = mybir.MatmulPerfMode.DoubleRow

B, N, DIN, U, FD, SD = 32, 2048, 2, 64, 32, 64
NCORES = 8
BL = B // NCORES          # batches per core
NTW = 512                 # n-tile width
NT = N // NTW             # 4 n-tiles
NBW = 128                 # node-block width
NB = N // NBW             # 16 node blocks
NP = NB // 2              # 8 node-block pairs (DoubleRow)
FROWS = DIN + U           # 66

SC_ADJ = 64.0             # host scale on adjacency (fp8)
SC_Y = 8.0                # host scale on y kernels (fp8 y tiles)
SC_ID = 512.0             # host scale on identity kernels (bf16)
GATE_SCALE = 0.25 / 512.0 # sigmoid/tanh pre-scale: mean over 4 supports / 512


def _build_program():
    nc = bacc.Bacc("TRN2", debug=False, num_devices=NCORES)

    d = {}

    def din(name, shape, dt):
        d[name] = nc.dram_tensor(name, shape, dt, kind="ExternalInput").ap()

    din("xT", [BL, FROWS, N], BF16)
    din("hT", [BL, U, N], F32)
    din("a1T", [N, N], FP8)
    din("a2T", [N, N], FP8)
    din("fsT", [FD + SD, N], F32)
    din("wq", [FD, U], F32)
    din("wk", [FD, U], F32)
    din("ws1", [FD + SD, U], F32)
    din("bs1v", [U, 1], F32)
    din("ws2", [U, 1], F32)
    din("bs2v", [1, 1], F32)
    din("kkall", [FROWS, 3 * 2 * U], BF16)
    din("kk0", [FROWS, 2 * U], BF16)
    din("kcall", [FROWS, 3 * U], BF16)
    din("kc0", [FROWS, U], BF16)
    din("bruh", [2 * U, 1], F32)
    din("bc2", [2 * U, 1], F32)
    din("ones8", [NBW, 2, NBW], FP8)        # constant 1/64
    out_h = nc.dram_tensor("out", [BL, U, N], F32, kind="ExternalOutput").ap()
    uscr = nc.dram_tensor("uscr", [BL, U, N], F32).ap()

    with tile.TileContext(nc) as tc, \
            nc.allow_low_precision(reason="fp8 support contraction by design"):
        _emit(tc, d, out_h, uscr)
    nc.compile()
    return nc


def _emit(tc, d, out_h, uscr):
    nc = tc.nc
    ctx = contextlib.ExitStack()
    const = ctx.enter_context(tc.tile_pool(name="const", bufs=1))
    persist = ctx.enter_context(tc.tile_pool(name="persist", bufs=1))
    adjp = ctx.enter_context(tc.tile_pool(name="adjp", bufs=4))
    ypool = ctx.enter_context(tc.tile_pool(name="ypool", bufs=1))
    stage = ctx.enter_context(tc.tile_pool(name="stage", bufs=2))
    p3p = ctx.enter_context(tc.tile_pool(name="p3p", bufs=2))
    psacc = ctx.enter_context(tc.tile_pool(name="psacc", bufs=3, space="PSUM"))
    psscr = ctx.enter_context(tc.tile_pool(name="psscr", bufs=3, space="PSUM"))
    pscs = ctx.enter_context(tc.tile_pool(name="pscs", bufs=2, space="PSUM"))

    # ---- constants / weights in SBUF ----
    def cload(name):
        ap = d[name]
        t = const.tile(list(ap.shape), ap.dtype, name=f"c_{name}")
        nc.sync.dma_start(out=t, in_=ap)
        return t

    fsT = const.tile([FD + SD, N], F32, name="c_fsT")
    nc.sync.dma_start(out=fsT[0:FD, :], in_=d["fsT"][0:FD, :])
    wq = cload("wq")
    wk = cload("wk")
    nc.sync.dma_start(out=fsT[FD:, :], in_=d["fsT"][FD:, :])
    ws1 = cload("ws1")
    bs1v = cload("bs1v")
    ws2 = cload("ws2")
    bs2v = cload("bs2v")

    kkall = cload("kkall")
    kcall = cload("kcall")
    kk0 = cload("kk0")
    kc0 = cload("kc0")
    bruh = cload("bruh")
    bc2 = cload("bc2")
    ones8 = cload("ones8")
    s8 = const.tile([NBW, 2, N], FP8, name="s8")
    nc.gpsimd.memset(s8, 0.0)
    one_bc = const.tile([128, NTW], F32, name="one_bc")
    nc.vector.memset(one_bc, 1.0)

    # ---- persistent activations ----
    xT = [persist.tile([FROWS, N], BF16, name=f"xT{b}", tag=f"xT{b}")
          for b in range(BL)]
    for b in range(BL):
        nc.sync.dma_start(out=xT[b], in_=d["xT"][b])

    QT = persist.tile([U // 2, 2, N], FP8, name="QT", tag="QT")
    KT = persist.tile([U // 2, 2, N], FP8, name="KT", tag="KT")
    # resident raw attention support e^T = exp(KQ^T/8), fp8
    et = [persist.tile([NBW, NB, NTW], FP8, name=f"et{t}", tag=f"et{t}")
          for t in range(NT)]
    # rdbc[t][p, n] = 64/d[n]: e-term normalizer, applied to PSUM e-partials
    rdbc = [persist.tile([NBW, NTW], F32, name=f"rdbc{t}", tag=f"rdbc{t}")
            for t in range(NT)]

    # ---- prelude: Q^T, K^T, s ----
    for t in range(NT):
        sl = slice(t * NTW, (t + 1) * NTW)
        # Q/K/s evacuations all on DVE (tensor_scalar relu): keeps the ACT
        # queue free for the 16 serial exp instructions that gate tile 0
        pq = psscr.tile([U, NTW], F32, name="pq", tag="scr")
        nc.tensor.matmul(pq, wq, fsT[0:FD, sl], start=True, stop=True)
        nc.vector.tensor_scalar(QT[:, 0, sl], pq[0:U // 2, :], 8.0, 0.0,
                                mybir.AluOpType.mult, mybir.AluOpType.max)
        nc.scalar.activation(QT[:, 1, sl], pq[U // 2:U, :], AF.Relu,
                             scale=8.0)
        pk = psscr.tile([U, NTW], F32, name="pk", tag="scr")
        nc.tensor.matmul(pk, wk, fsT[0:FD, sl], start=True, stop=True)
        nc.vector.tensor_scalar(KT[:, 0, sl], pk[0:U // 2, :], 8.0, 0.0,
                                mybir.AluOpType.mult, mybir.AluOpType.max)
        nc.scalar.activation(KT[:, 1, sl], pk[U // 2:U, :], AF.Relu,
                             scale=8.0)
        ps1 = psscr.tile([U, NTW], F32, name="ps1", tag="scr")
        nc.tensor.matmul(ps1, ws1, fsT[:, sl], start=True, stop=True)
        s1t = stage.tile([U, NTW], F32, name="s1t", tag="sig")
        nc.vector.tensor_scalar(s1t, ps1, bs1v, 0.0,
                                mybir.AluOpType.add, mybir.AluOpType.max)
        ps2 = psscr.tile([1, NTW], F32, name="ps2", tag="scr")
        nc.tensor.matmul(ps2, ws2, s1t, start=True, stop=True)
        nc.vector.tensor_scalar(s8[0:1, 0, sl], ps2, bs2v, 0.0,
                                mybir.AluOpType.add, mybir.AluOpType.max)

    # ---- e-generation chain (per n-tile t), emitted as thunks ----
    def eg_thunks(t):
        """raw e^T tile: et[t][:, j, :] = exp(K Q^T / 8) as fp8."""
        sl = slice(t * NTW, (t + 1) * NTW)

        def mk(j):
            def f():
                pe = psscr.tile([NBW, NTW], F32, name="pe", tag="scr")
                nc.tensor.matmul(pe, KT[:, :, j * NBW:(j + 1) * NBW],
                                 QT[:, :, sl], start=True, stop=True,
                                 perf_mode=DR)
                nc.scalar.activation(et[t][:, j, :], pe, AF.Exp,
                                     scale=0.125 / 64.0)
            return f
        return [mk(j) for j in range(NB)]

    def colsum_thunks(t):
        """rdbc[t][p, n] = 64 / (s[n] + colsum(e^T)[n]), all partitions."""
        sl = slice(t * NTW, (t + 1) * NTW)
        pcs = pscs.tile([NBW, NTW], F32, name="pcs", tag="pcs")
        th = []
        for k in range(NP):
            def f(k=k):
                nc.tensor.matmul(pcs, ones8, et[t][:, 2 * k:2 * k + 2, :],
                                 start=(k == 0), stop=False, perf_mode=DR)
            th.append(f)
        th.append(lambda: nc.tensor.matmul(
            pcs, ones8, s8[:, :, sl], start=False, stop=True, perf_mode=DR))
        th.append(lambda: nc.vector.reciprocal(rdbc[t], pcs))
        return th

    # ---- phase-1 Y tiles: Y[m,b] = X_b @ [8*k_r[m]|8*k_u[m]] ----
    # stored fp8 [node%128, node//128, m, u']; copies split DVE/ACT
    y = [ypool.tile([NBW, NB, 3, 2 * U], FP8, name=f"y_{b}", tag=f"y{b}")
         for b in range(BL)]

    def ygen_thunks(b):
        def mk(j):
            def f():
                nsl = slice(j * NBW, (j + 1) * NBW)
                py = psscr.tile([NBW, 3 * 2 * U], F32, name="py", tag="scr")
                nc.tensor.matmul(py, xT[b][:, nsl], kkall, start=True,
                                 stop=True)
                if j % 2 == 0:
                    nc.vector.tensor_copy(
                        y[b][:, j, :, :],
                        py.rearrange("p (m u) -> p m u", m=3))
                else:
                    nc.scalar.activation(
                        y[b][:, j, :, :].rearrange("p m u -> p (m u)"),
                        py, AF.Copy)
            return f
        return [mk(j) for j in range(NB)]

    def interleave(main, extra, ratio=2):
        mi = ei = 0
        while mi < len(main) or ei < len(extra):
            for _ in range(ratio):
                if mi < len(main):
                    main[mi](); mi += 1
            if ei < len(extra):
                extra[ei](); ei += 1


    _adj_cache = {}

    def adjslice(name, t):
        # persist: each slice is DMA'd once (phase 1) and reused in phase 2
        if (name, t) not in _adj_cache:
            sl = d[name][:, t * NTW:(t + 1) * NTW]
            a = persist.tile([NBW, NB, NTW], FP8, name=f"{name}_{t}",
                             tag=f"{name}_{t}")
            nc.sync.dma_start(out=a,
                              in_=sl.rearrange("(j p) w -> p j w", p=NBW))
            _adj_cache[(name, t)] = a
        return _adj_cache[(name, t)]

    # =================== phase 1: r & u gates ===================
    # Each gate group: e-support contraction first (own PSUM group), then a
    # DVE multiply by rdbc[t] on the PSUM partial, then identity + adjacency
    # terms accumulate on top (start=False).
    def e_thunks(yt, t, pa):
        th = []
        for k in range(NP):
            def f(k=k):
                nc.tensor.matmul(pa, yt[:, 2 * k:2 * k + 2, 2, :],
                                 et[t][:, 2 * k:2 * k + 2, :],
                                 start=(k == 0), stop=(k == NP - 1),
                                 perf_mode=DR)
            th.append(f)
        return th

    def rest_thunks1(b, t, sl, a1, a2, pa):
        th = [lambda: nc.vector.tensor_mul(pa, pa, rdbc[t]),
              lambda: nc.tensor.matmul(pa, kk0, xT[b][:, sl], start=False,
                                       stop=False, skip_group_check=True)]
        for m, mov in ((0, a1), (1, a2)):
            for k in range(NP):
                def f(m=m, mov=mov, k=k):
                    nc.tensor.matmul(pa, y[b][:, 2 * k:2 * k + 2, m, :],
                                     mov[:, 2 * k:2 * k + 2, :],
                                     start=False,
                                     stop=(m == 1 and k == NP - 1),
                                     perf_mode=DR, skip_group_check=True)
                th.append(f)
        return th

    def gate1(b, t, sl, pa):
        def f():
            # sigmoid(z) computed as 0.5 + 0.5*tanh(z/2): keeps every ACT
            # function in the exp_and_others table (no table reloads).
            # sig holds T = tanh(z/2) = 2*sigmoid(z) - 1.
            sig = stage.tile([128, NTW], F32, name="sig", tag="sig")
            nc.scalar.activation(sig, pa, AF.Tanh, scale=GATE_SCALE / 2,
                                 bias=bruh)
            # rows 0:64 in place: (T_r + 1)*h = 2*r*h; the c-pass kernels'
            # h-rows carry a compensating 1/2. u kept as raw T_u.
            rtmp = stage.tile([U, NTW], F32, name="rtmp", tag="rtmp")
            nc.gpsimd.tensor_add(rtmp, sig[0:U, :], one_bc[0:U, :])
            nc.gpsimd.tensor_mul(xT[b][0:U, sl], rtmp, xT[b][0:U, sl])
            nc.sync.dma_start(out=uscr[b][:, sl], in_=sig[U:128, :])
        return [f]

    yc = [None, None]

    def ycgen_thunks(yct, p):
        th = []
        for half in range(2):
            b = 2 * p + half
            usl = slice(half * U, (half + 1) * U)
            for j in range(NB):
                def f(b=b, usl=usl, j=j, yct=yct, half=half):
                    nsl = slice(j * NBW, (j + 1) * NBW)
                    pyc = psscr.tile([NBW, 3 * U], F32, name="pyc", tag="scr")
                    nc.tensor.matmul(pyc, xT[b][:, nsl], kcall,
                                     start=True, stop=True)
                    dst = yct[:, j, :, usl]
                    src = pyc.rearrange("p (m u) -> p m u", m=3)
                    if half == 0:
                        nc.vector.tensor_copy(dst, src)
                    else:
                        nc.scalar.activation(dst, src, AF.Copy)
                th.append(f)
        return th

    # prologue: e-gen + colsum for tile 0; adj preload
    for f in eg_thunks(0) + colsum_thunks(0):
        f()
    preload = (adjslice("a1T", 0), adjslice("a2T", 0))

    for t in range(NT):
        sl = slice(t * NTW, (t + 1) * NTW)
        if t == 0:
            a1, a2 = preload
        else:
            a1 = adjslice("a1T", t)
            a2 = adjslice("a2T", t)
        pa = [psacc.tile([128, NTW], F32, name="pa", tag="acc")
              for _ in range(BL)]
        # schedule: e-streams lead their group's rest by one slot so the
        # DVE rdbc-multiply hides under the next group's e-stream.
        main = []
        if t == 0:
            # y generation woven in so each y[b] is ready just before use
            main += ygen_thunks(0) + ygen_thunks(1)
            main += e_thunks(y[0], t, pa[0])
            main += ygen_thunks(2)
            main += e_thunks(y[1], t, pa[1])
            main += rest_thunks1(0, t, sl, a1, a2, pa[0]) + gate1(0, t, sl, pa[0])
            main += ygen_thunks(3)
            main += e_thunks(y[2], t, pa[2])
            main += rest_thunks1(1, t, sl, a1, a2, pa[1]) + gate1(1, t, sl, pa[1])
            main += e_thunks(y[3], t, pa[3])
            main += rest_thunks1(2, t, sl, a1, a2, pa[2]) + gate1(2, t, sl, pa[2])
            main += rest_thunks1(3, t, sl, a1, a2, pa[3]) + gate1(3, t, sl, pa[3])
        else:
            main += e_thunks(y[0], t, pa[0]) + e_thunks(y[1], t, pa[1])
            main += e_thunks(y[2], t, pa[2])
            main += rest_thunks1(0, t, sl, a1, a2, pa[0]) + gate1(0, t, sl, pa[0])
            main += e_thunks(y[3], t, pa[3])
            main += rest_thunks1(1, t, sl, a1, a2, pa[1]) + gate1(1, t, sl, pa[1])
        post = []
        if t > 0:
            post += rest_thunks1(2, t, sl, a1, a2, pa[2]) + gate1(2, t, sl, pa[2])
            post += rest_thunks1(3, t, sl, a1, a2, pa[3]) + gate1(3, t, sl, pa[3])
        extra = eg_thunks(t + 1) if t + 1 < NT else []
        post_extra = []
        if t == NT - 1:
            # yc[0] (batches 0/1) reads post-rh xT: only legal after gate1(1)
            yc[0] = ypool.tile([NBW, NB, 3, 2 * U], FP8, name="yc_0",
                               tag="y0")
            post_extra = ycgen_thunks(yc[0], 0)
        allm = main + post
        alle = extra
        if post_extra:
            interleave(main, extra, ratio=max(1, len(main) // max(1, len(extra))))
            interleave(post, post_extra, ratio=1)
        else:
            interleave(allm, alle, ratio=max(1, len(allm) // max(1, len(alle))))
        # colsum for t+1 at tile end: its exp deps had the whole tile to run
        if t + 1 < NT:
            for f in colsum_thunks(t + 1):
                f()

    # =================== phase 2+3: c gate & h_new ===================
    yc[1] = ypool.tile([NBW, NB, 3, 2 * U], FP8, name="yc_1", tag="y1")
    a1 = adjslice("a1T", 0)
    a2 = adjslice("a2T", 0)
    for f in ycgen_thunks(yc[1], 1):
        f()

    def hu_prefetch(p, t, sl, hu):
        # single 2-batch DMA each for h and u: [2, 64, w] -> [128, w]
        b0 = 2 * p

        def f():
            hp = p3p.tile([128, NTW], F32, name="hp", tag="hp")
            up = p3p.tile([128, NTW], F32, name="up", tag="up")
            nc.sync.dma_start(
                out=hp, in_=d["hT"][b0:b0 + 2, :, sl].rearrange(
                    "b u w -> (b u) w"))
            nc.sync.dma_start(
                out=up, in_=uscr[b0:b0 + 2, :, sl].rearrange(
                    "b u w -> (b u) w"))
            hu.extend((hp, up))
        return [f]

    def rest_thunks2(p, t, sl, a1, a2, pa, hu):
        b0, b1 = 2 * p, 2 * p + 1

        th = [lambda: nc.vector.tensor_mul(pa, pa, rdbc[t]),
              lambda: nc.tensor.matmul(pa[0:U, :], kc0, xT[b0][:, sl],
                                       start=False, stop=False,
                                       skip_group_check=True),
              lambda: nc.tensor.matmul(pa[U:128, :], kc0, xT[b1][:, sl],
                                       start=False, stop=False,
                                       skip_group_check=True)]
        for m, mov in ((0, a1), (1, a2)):
            for k in range(NP):
                def f(m=m, mov=mov, k=k):
                    nc.tensor.matmul(pa, yc[p][:, 2 * k:2 * k + 2, m, :],
                                     mov[:, 2 * k:2 * k + 2, :],
                                     start=False,
                                     stop=(m == 1 and k == NP - 1),
                                     perf_mode=DR, skip_group_check=True)
                th.append(f)
        return th

    def tail2(p, t, sl, pa, hu, splits=2):
        b0, b1 = 2 * p, 2 * p + 1

        def f():
            hp, up = hu
            ct = stage.tile([128, NTW], F32, name="ct", tag="sig")
            t1 = p3p.tile([128, NTW], F32, name="t1", tag="t1")
            # compute in column chunks so ACT/Pool/DVE stages pipeline, but
            # write out full-width (HWDGE descriptor slots are the scarce
            # resource at the drain)
            for c0 in range(0, NTW, NTW // splits):
                cs = slice(c0, c0 + NTW // splits)
                nc.scalar.activation(ct[:, cs], pa[:, cs], AF.Tanh,
                                     scale=GATE_SCALE, bias=bc2)
                # h_new = c + u*(h-c) with up = T_u = 2u-1:
                # t1 = h-c (Pool); t1 = (T_u+1)*t1; t1 = t1/2 + c (DVE)
                nc.gpsimd.tensor_sub(t1[:, cs], hp[:, cs], ct[:, cs])
                nc.vector.scalar_tensor_tensor(
                    t1[:, cs], up[:, cs], 1.0, t1[:, cs],
                    mybir.AluOpType.add, mybir.AluOpType.mult)
                nc.vector.scalar_tensor_tensor(
                    t1[:, cs], t1[:, cs], 0.5, ct[:, cs],
                    mybir.AluOpType.mult, mybir.AluOpType.add)
                if splits == 4 and c0 + NTW // splits in (NTW // 2, NTW):
                    hs = slice(c0 + NTW // splits - NTW // 2,
                               c0 + NTW // splits)
                    nc.sync.dma_start(
                        out=out_h[b0:b0 + 2, :,
                                  t * NTW + hs.start:t * NTW + hs.stop]
                        .rearrange("b u w -> (b u) w"),
                        in_=t1[:, hs])
            if splits != 4:
                nc.sync.dma_start(
                    out=out_h[b0:b0 + 2, :, sl].rearrange("b u w -> (b u) w"),
                    in_=t1)
        return [f]

    prev = []
    for t in range(NT):
        sl = slice(t * NTW, (t + 1) * NTW)
        if t > 0:
            a1 = adjslice("a1T", t)
            a2 = adjslice("a2T", t)
        pa = [psacc.tile([128, NTW], F32, name="pa2", tag="acc")
              for _ in range(BL // 2)]
        hu0, hu1 = [], []
        main = hu_prefetch(0, t, sl, hu0) + prev + hu_prefetch(1, t, sl, hu1)
        main += e_thunks(yc[0], t, pa[0]) + e_thunks(yc[1], t, pa[1])
        main += rest_thunks2(0, t, sl, a1, a2, pa[0], hu0) \
            + tail2(0, t, sl, pa[0], hu0)
        for f in main:
            f()
        # carry p=1's rest into the next tile so its DVE multiply and the
        # tail chain overlap with the next tile's e-streams
        prev = rest_thunks2(1, t, sl, a1, a2, pa[1], hu1) \
            + tail2(1, t, sl, pa[1], hu1, splits=4 if t == NT - 1 else 2)
    for f in prev:
        f()

    ctx.close()


_CACHE = {}


def _get_program():
    if "nc" not in _CACHE:
        _CACHE["nc"] = _build_program()
    return _CACHE["nc"]


def _prep_inputs(inputs, h_prev, adj1, adj2, feat, SE, Wq, Wk, Ws1, bs1, Ws2,
                 bs2, r_kernel, r_bias, u_kernel, u_bias, c_kernel, c_bias):
    bf = ml_dtypes.bfloat16
    f8 = ml_dtypes.float8_e4m3fn
    f32 = np.float32
    perm = list(range(DIN, FROWS)) + list(range(DIN))  # [h(64); inputs(2)]

    h3 = np.asarray(h_prev, f32).reshape(B, N, U)
    hT = np.ascontiguousarray(h3.transpose(0, 2, 1))            # [B, U, N]
    inT = np.asarray(inputs, f32).transpose(0, 2, 1)            # [B, DIN, N]
    xT = np.concatenate([hT, inT], axis=1).astype(bf)           # [B, 66, N]

    rk = np.asarray(r_kernel, f32)[:, perm, :]
    uk = np.asarray(u_kernel, f32)[:, perm, :]
    ck = np.asarray(c_kernel, f32)[:, perm, :]
    kkall = np.concatenate(
        [np.concatenate([rk[m], uk[m]], axis=1) for m in (1, 2, 3)],
        axis=1) * SC_Y                                          # [66, 384]
    kk0 = np.concatenate([rk[0], uk[0]], axis=1) * SC_ID        # [66, 128]
    # c-pass x_cat holds 2*r*h in its h-rows (r comes from the tanh-form
    # sigmoid as T_r + 1); compensate with a 1/2 on the kernels' h-rows.
    ck = ck.copy()
    ck[:, 0:U, :] *= 0.5
    kcall = np.concatenate([ck[1], ck[2], ck[3]], axis=1) * SC_Y
    kc0 = ck[0] * SC_ID

    shared = {
        "a1T": np.ascontiguousarray(np.asarray(adj1, f32).T * SC_ADJ).astype(f8),
        "a2T": np.ascontiguousarray(np.asarray(adj2, f32).T * SC_ADJ).astype(f8),
        "fsT": np.ascontiguousarray(
            np.concatenate([np.asarray(feat, f32).T, np.asarray(SE, f32).T],
                           axis=0)),
        "wq": np.asarray(Wq, f32),
        "wk": np.asarray(Wk, f32),
        "ws1": np.asarray(Ws1, f32),
        "bs1v": np.asarray(bs1, f32).reshape(U, 1),
        "ws2": np.asarray(Ws2, f32).reshape(U, 1),
        "bs2v": np.asarray(bs2, f32).reshape(1, 1),
        "kkall": kkall.astype(bf),
        "kk0": kk0.astype(bf),
        "kcall": kcall.astype(bf),
        "kc0": kc0.astype(bf),
        "bruh": np.concatenate([np.asarray(r_bias, f32).mean(0),
                                np.asarray(u_bias, f32).mean(0)]
                               ).reshape(-1, 1) * 0.5,
        "bc2": np.tile(np.asarray(c_bias, f32).mean(0), 2).reshape(-1, 1),
        "ones8": np.full((NBW, 2, NBW), 1.0 / 64.0, f8),
    }
    in_maps = []
    for c in range(NCORES):
        bsl = slice(c * BL, (c + 1) * BL)
        m = dict(shared)
        m["xT"] = np.ascontiguousarray(xT[bsl])
        m["hT"] = np.ascontiguousarray(hT[bsl])
        in_maps.append(m)
    return in_maps


def kernel(**inputs):
    os.environ.setdefault("NEURON_RT_RESET_CORES", "1")
    nc = _get_program()
    in_maps = _prep_inputs(**inputs)
    res = None
    err = None
    for _ in range(2):
        try:
            res = run_bass_kernel_spmd(nc, in_maps, list(range(NCORES)))
            break
        except Exception as e:  # e.g. a wedged device; retry once
            err = e
    if res is None:
        raise err
    outs = []
    for c in range(NCORES):
        o = res.results[c]["out"]                     # [BL, U, N] f32
        outs.append(o.transpose(0, 2, 1).reshape(BL, N * U))
    return np.concatenate(outs, axis=0).astype(np.float32)
